# revision 5
# baseline (speedup 1.0000x reference)
"""3x3 valid conv (cross-correlation) of an 8192x8192 fp32 image on 8 TRN2 NeuronCores.

Strategy
--------
Output rows are sharded across 8 cores. Each core computes 8 full 126-row
"band blocks" (1008 rows, out rows [i*1008, i*1008+1008)), and the leftover
126-row slab (out rows 8064..8189) is split BY WIDTH across the cores
(~1024 columns each) so no core runs a mostly-empty rump block. Every core
receives its input rows/cols WITH the 2-element halo already included, so
no on-device collectives are needed.

Per core, the conv runs on the TensorEngine as banded matmuls: for a block
of 128 input rows, out[o, c] += sum_p band_d[p, o] * x[p, c+d] where
band_d[p, o] = w[p-o, d] (3 diagonals). The 3 column taps d=0,1,2 are 3
matmuls over column-shifted views of the same SBUF tile, accumulated in
PSUM. 126 output rows are produced per 128-row block.

Precision: the tolerance gate is rel_err < 2e-2, so the whole pipeline runs
in fp16 (10 mantissa bits; x ~ N(0,1) and |y| < ~15 are far inside fp16
range). The host casts x/w to fp16, the PE runs fp16 matmuls (1 cycle per
moving column — 3 matmuls per output tile instead of the 9 an fp32r
precision-split needs), PSUM accumulates in fp32, and the ScalarEngine adds
the bias while downcasting to fp16 for the store. End-to-end error is
~8e-4. HBM traffic halves vs fp32 (2B/elem in + 2B/elem out), which is the
memory roofline of the problem.
"""
import numpy as np

H = 8192
W = 8192
OH = H - 2
OW = W - 2
NCORES = 8
BLK_OUT = 126
NBLK = 8  # full band blocks per core
RPC = NBLK * BLK_OUT  # 1008 contiguous output rows per core
IN_ROWS = RPC + 2  # 1010 input rows per core shard
WT = 512  # PSUM bank free dim (fp32): 15 full tiles + one 510 tile = 8190
NWT = (OW + WT - 1) // WT  # 16
LDC = 2048  # input-load DMA chunk (cols)
STC = 2048  # output-store DMA chunk (cols)
# leftover slab: out rows [8064, 8190) split by width across cores
SLAB_R0 = NCORES * RPC  # 8064
SLAB_OC = 1024  # slab output cols per core (core 7: only 1022 valid)
SLAB_IC = SLAB_OC + 2

_cache = {}


def _build(reps=1):
    from contextlib import ExitStack

    import concourse.bacc as bacc
    import concourse.tile as tile
    import concourse.mybir as mybir

    f32 = mybir.dt.float32
    f16 = mybir.dt.float16
    nc = bacc.Bacc("TRN2", target_bir_lowering=False, debug=False)
    xs = nc.dram_tensor("xs", [IN_ROWS, W], f16, kind="ExternalInput")
    xs2 = nc.dram_tensor("xs2", [128, SLAB_IC], f16, kind="ExternalInput")
    wb = nc.dram_tensor("wb", [128, 378], f16, kind="ExternalInput")
    bc = nc.dram_tensor("bc", [128, 1], f32, kind="ExternalInput")
    ys = nc.dram_tensor("ys", [RPC, OW], f16, kind="ExternalOutput")
    ys2 = nc.dram_tensor("ys2", [BLK_OUT, SLAB_OC], f16, kind="ExternalOutput")
    with tile.TileContext(nc) as tc:
        with (
            tc.tile_pool(name="wpool", bufs=1) as wpool,
            tc.tile_pool(name="xraw", bufs=5) as xraw,
            tc.tile_pool(name="yout", bufs=4) as yout,
            tc.tile_pool(name="psum", bufs=8, space="PSUM") as psum,
            ExitStack() as rep_ctx,
        ):
            wt = wpool.tile([128, 378], f16)
            nc.sync.dma_start(wt[:], wb[:])
            bt = wpool.tile([128, 1], f32)
            nc.sync.dma_start(bt[:], bc[:])
            if reps > 1:
                # timing-only variant: repeat the body on-device so per-
                # iteration device time can be isolated from the (large)
                # axon dispatch overhead
                rep_ctx.enter_context(tc.For_i(0, reps, 1))

            def do_block(src, src_r0, irows, dst, dst_r0, ocols):
                """One 126-row band block: chunked load, 3 matmuls per
                512-col tile, PSUM drain alternating ScalarE/VectorE,
                chunked store. Chunking keeps the (single, pooled) DMA
                path interleaved so the PE never starves and cools."""
                icols = ocols + 2
                xr = xraw.tile([128, W], f16, tag="xr")
                for c0 in range(0, icols, LDC):
                    cw = min(LDC, icols - c0)
                    nc.sync.dma_start(
                        xr[:irows, c0 : c0 + cw],
                        src[src_r0 : src_r0 + irows, c0 : c0 + cw],
                    )
                yo = yout.tile([126, OW], f16, tag="yo")
                ntl = (ocols + WT - 1) // WT
                for t in range(ntl):
                    c0 = t * WT
                    cw = min(WT, ocols - c0)
                    pst = psum.tile([126, WT], f32, tag="ps")
                    for d in range(3):
                        nc.tensor.matmul(
                            pst[:BLK_OUT, :cw],
                            wt[:irows, d * 126 : d * 126 + BLK_OUT],
                            xr[:irows, c0 + d : c0 + d + cw],
                            start=(d == 0),
                            stop=(d == 2),
                        )
                    if t % 2 == 0:
                        nc.scalar.activation(
                            yo[:BLK_OUT, c0 : c0 + cw],
                            pst[:BLK_OUT, :cw],
                            mybir.ActivationFunctionType.Identity,
                            bias=bt[:BLK_OUT, :],
                            scale=1.0,
                        )
                    else:
                        nc.vector.tensor_scalar_add(
                            yo[:BLK_OUT, c0 : c0 + cw],
                            pst[:BLK_OUT, :cw],
                            bt[:BLK_OUT, :],
                        )
                for c0 in range(0, ocols, STC):
                    cw = min(STC, ocols - c0)
                    nc.sync.dma_start(
                        dst[dst_r0 : dst_r0 + BLK_OUT, c0 : c0 + cw],
                        yo[:BLK_OUT, c0 : c0 + cw],
                    )

            # leftover slab first: its small transfers prime the pipe and
            # keep the tail of the kernel on the big, well-overlapped blocks
            do_block(xs2, 0, 128, ys2, 0, SLAB_OC)
            for j in range(NBLK):
                do_block(xs, j * BLK_OUT, 128, ys, j * BLK_OUT, OW)
    nc.compile()
    return nc


def _get_nc():
    if "nc" not in _cache:
        _cache["nc"] = _build()
    return _cache["nc"]


def make_inputs(x, weight, bias):
    """Host-side shard/prep: per-core input maps for run_bass_kernel_spmd."""
    x = np.asarray(x, np.float32).astype(np.float16)
    w = np.asarray(weight, np.float32).astype(np.float16)
    wbm = np.zeros((128, 378), np.float16)
    o = np.arange(BLK_OUT)
    for d in range(3):
        for k in range(3):
            wbm[o + k, d * BLK_OUT + o] = w[k, d]
    bcm = np.full((128, 1), np.float32(np.asarray(bias).reshape(-1)[0]), np.float32)
    in_maps = []
    for i in range(NCORES):
        xs2 = np.zeros((128, SLAB_IC), np.float16)
        c0 = i * SLAB_OC
        c1 = min(c0 + SLAB_IC, W)
        xs2[:, : c1 - c0] = x[SLAB_R0 : SLAB_R0 + 128, c0:c1]
        in_maps.append(
            {
                "xs": x[i * RPC : i * RPC + IN_ROWS],
                "xs2": xs2,
                "wb": wbm,
                "bc": bcm,
            }
        )
    return in_maps


def kernel(x, weight, bias):
    from concourse.bass_utils import run_bass_kernel_spmd

    nc = _get_nc()
    in_maps = make_inputs(x, weight, bias)
    res = run_bass_kernel_spmd(nc, in_maps, list(range(NCORES)))
    out = np.empty((OH, OW), np.float32)
    for i in range(NCORES):
        out[i * RPC : (i + 1) * RPC] = res.results[i]["ys"]
        c0 = i * SLAB_OC
        c1 = min(c0 + SLAB_OC, OW)
        out[SLAB_R0:OH, c0:c1] = res.results[i]["ys2"][:, : c1 - c0]
    return out


# revision 6
# speedup vs baseline: 1.0415x; 1.0415x over previous
"""3x3 valid conv (cross-correlation) of an 8192x8192 fp32 image on 8 TRN2 NeuronCores.

Strategy
--------
Output rows are sharded across 8 cores. Each core computes 8 full 126-row
"band blocks" (1008 rows, out rows [i*1008, i*1008+1008)), and the leftover
126-row slab (out rows 8064..8189) is split BY WIDTH across the cores
(~1024 columns each) so no core runs a mostly-empty rump block. Every core
receives its input rows/cols WITH the 2-element halo already included, so
no on-device collectives are needed.

Per core, the conv runs on the TensorEngine as banded matmuls: for a block
of 128 input rows, out[o, c] += sum_p band_d[p, o] * x[p, c+d] where
band_d[p, o] = w[p-o, d] (3 diagonals). The 3 column taps d=0,1,2 are 3
matmuls over column-shifted views of the same SBUF tile, accumulated in
PSUM. 126 output rows are produced per 128-row block.

Precision: the tolerance gate is rel_err < 2e-2, so the whole pipeline runs
in fp16 (10 mantissa bits; x ~ N(0,1) and |y| < ~15 are far inside fp16
range). The host casts x/w to fp16, the PE runs fp16 matmuls (1 cycle per
moving column — 3 matmuls per output tile instead of the 9 an fp32r
precision-split needs), PSUM accumulates in fp32, and the ScalarEngine adds
the bias while downcasting to fp16 for the store. End-to-end error is
~8e-4. HBM traffic halves vs fp32 (2B/elem in + 2B/elem out), which is the
memory roofline of the problem.
"""
import numpy as np

H = 8192
W = 8192
OH = H - 2
OW = W - 2
NCORES = 8
BLK_OUT = 126
NBLK = 8  # full band blocks per core
RPC = NBLK * BLK_OUT  # 1008 contiguous output rows per core
IN_ROWS = RPC + 2  # 1010 input rows per core shard
WT = 512  # PSUM bank free dim (fp32): 15 full tiles + one 510 tile = 8190
NWT = (OW + WT - 1) // WT  # 16
LDC = 2048  # input-load DMA chunk (cols)
STC = 2048  # output-store DMA chunk (cols)
# leftover slab: out rows [8064, 8190) split by width across cores
SLAB_R0 = NCORES * RPC  # 8064
SLAB_OC = 1024  # slab output cols per core (core 7: only 1022 valid)
SLAB_IC = SLAB_OC + 2

_cache = {}


def _build(reps=1):
    from contextlib import ExitStack

    import concourse.bacc as bacc
    import concourse.tile as tile
    import concourse.mybir as mybir

    f32 = mybir.dt.float32
    f16 = mybir.dt.float16
    nc = bacc.Bacc("TRN2", target_bir_lowering=False, debug=False)
    xs = nc.dram_tensor("xs", [IN_ROWS, W], f16, kind="ExternalInput")
    xs2 = nc.dram_tensor("xs2", [128, SLAB_IC], f16, kind="ExternalInput")
    wb = nc.dram_tensor("wb", [128, 378], f16, kind="ExternalInput")
    bc = nc.dram_tensor("bc", [128, 1], f32, kind="ExternalInput")
    ys = nc.dram_tensor("ys", [RPC, OW], f16, kind="ExternalOutput")
    ys2 = nc.dram_tensor("ys2", [BLK_OUT, SLAB_OC], f16, kind="ExternalOutput")
    with tile.TileContext(nc) as tc:
        with (
            tc.tile_pool(name="wpool", bufs=1) as wpool,
            tc.tile_pool(name="xraw", bufs=5) as xraw,
            tc.tile_pool(name="yout", bufs=4) as yout,
            tc.tile_pool(name="psum", bufs=8, space="PSUM") as psum,
            ExitStack() as rep_ctx,
        ):
            wt = wpool.tile([128, 378], f16)
            nc.sync.dma_start(wt[:], wb[:])
            bt = wpool.tile([128, 1], f32)
            nc.sync.dma_start(bt[:], bc[:])
            if reps > 1:
                # timing-only variant: repeat the body on-device so per-
                # iteration device time can be isolated from the (large)
                # axon dispatch overhead
                rep_ctx.enter_context(tc.For_i(0, reps, 1))

            # Work list: leftover slab first (its small transfers prime the
            # pipe), then the 8 full-width band blocks.
            blocks = [(xs2, 0, ys2, 0, SLAB_OC)] + [
                (xs, j * BLK_OUT, ys, j * BLK_OUT, OW) for j in range(NBLK)
            ]

            def load_block(idx):
                """Chunked load of block idx into a fresh x tile."""
                src, src_r0, _, _, ocols = blocks[idx]
                icols = ocols + 2
                xr = xraw.tile([128, W], f16, tag="xr")
                for c0 in range(0, icols, LDC):
                    cw = min(LDC, icols - c0)
                    nc.sync.dma_start(
                        xr[:128, c0 : c0 + cw],
                        src[src_r0 : src_r0 + 128, c0 : c0 + cw],
                    )
                return xr

            def compute_store_block(idx, xr, last):
                """3 matmuls per 512-col tile, PSUM drain alternating
                ScalarE/VectorE, chunked store."""
                _, _, dst, dst_r0, ocols = blocks[idx]
                yo = yout.tile([126, OW], f16, tag="yo")
                ntl = (ocols + WT - 1) // WT
                for t in range(ntl):
                    c0 = t * WT
                    cw = min(WT, ocols - c0)
                    pst = psum.tile([126, WT], f32, tag="ps")
                    for d in range(3):
                        nc.tensor.matmul(
                            pst[:BLK_OUT, :cw],
                            wt[:128, d * 126 : d * 126 + BLK_OUT],
                            xr[:128, c0 + d : c0 + d + cw],
                            start=(d == 0),
                            stop=(d == 2),
                        )
                    if t % 2 == 0:
                        nc.scalar.activation(
                            yo[:BLK_OUT, c0 : c0 + cw],
                            pst[:BLK_OUT, :cw],
                            mybir.ActivationFunctionType.Identity,
                            bias=bt[:BLK_OUT, :],
                            scale=1.0,
                        )
                    else:
                        nc.vector.tensor_scalar_add(
                            yo[:BLK_OUT, c0 : c0 + cw],
                            pst[:BLK_OUT, :cw],
                            bt[:BLK_OUT, :],
                        )
                # small chunks on the final block so the pipeline-drain tail
                # exposes only one short store
                stc = 512 if last else STC
                for c0 in range(0, ocols, stc):
                    cw = min(stc, ocols - c0)
                    nc.sync.dma_start(
                        dst[dst_r0 : dst_r0 + BLK_OUT, c0 : c0 + cw],
                        yo[:BLK_OUT, c0 : c0 + cw],
                    )

            # Software pipeline with loads issued PF blocks ahead of the
            # stores, so a store chunk waiting at a DGE ring head for its
            # PSUM drain never starves the DMA engines of ready loads.
            PF = 2
            xtiles = {i: load_block(i) for i in range(min(PF + 1, len(blocks)))}
            for i in range(len(blocks)):
                if i + PF + 1 < len(blocks):
                    xtiles[i + PF + 1] = load_block(i + PF + 1)
                compute_store_block(i, xtiles.pop(i), last=(i == len(blocks) - 1))
    nc.compile()
    return nc


def _get_nc():
    if "nc" not in _cache:
        _cache["nc"] = _build()
    return _cache["nc"]


def make_inputs(x, weight, bias):
    """Host-side shard/prep: per-core input maps for run_bass_kernel_spmd."""
    x = np.asarray(x, np.float32).astype(np.float16)
    w = np.asarray(weight, np.float32).astype(np.float16)
    wbm = np.zeros((128, 378), np.float16)
    o = np.arange(BLK_OUT)
    for d in range(3):
        for k in range(3):
            wbm[o + k, d * BLK_OUT + o] = w[k, d]
    bcm = np.full((128, 1), np.float32(np.asarray(bias).reshape(-1)[0]), np.float32)
    in_maps = []
    for i in range(NCORES):
        xs2 = np.zeros((128, SLAB_IC), np.float16)
        c0 = i * SLAB_OC
        c1 = min(c0 + SLAB_IC, W)
        xs2[:, : c1 - c0] = x[SLAB_R0 : SLAB_R0 + 128, c0:c1]
        in_maps.append(
            {
                "xs": x[i * RPC : i * RPC + IN_ROWS],
                "xs2": xs2,
                "wb": wbm,
                "bc": bcm,
            }
        )
    return in_maps


def kernel(x, weight, bias):
    from concourse.bass_utils import run_bass_kernel_spmd

    nc = _get_nc()
    in_maps = make_inputs(x, weight, bias)
    res = run_bass_kernel_spmd(nc, in_maps, list(range(NCORES)))
    out = np.empty((OH, OW), np.float32)
    for i in range(NCORES):
        out[i * RPC : (i + 1) * RPC] = res.results[i]["ys"]
        c0 = i * SLAB_OC
        c1 = min(c0 + SLAB_OC, OW)
        out[SLAB_R0:OH, c0:c1] = res.results[i]["ys2"][:, : c1 - c0]
    return out
